# revision 14
# baseline (speedup 1.0000x reference)
"""Trainium2 Bass kernel for gated GQA attention (nn_Attention_6476810683032).

Sharding: 8 cores = 2 (batch DP) x 4 (head-group TP).
Core c handles batch b=c//4, head group g=c%4 (q-heads 4g..4g+3, kv-head g).
Each core computes a partial o_proj output [D, T] (its 4 heads' contribution,
transposed layout); the host sums the 4 partials per batch and transposes.

On-device per core (all matmuls bf16 with fp32 PSUM accumulation):
  - projections from host-pre-transposed hidden_t [D, T] (channel-major
    outputs for q/gate/k, token-major for v) -- no on-device transposes
  - RMS norm via ones-matmul partition reduction; the rsqrt row is
    partition-broadcast on the Pool engine (not a K=1 PE matmul) and the
    weight multiply is fused into one scalar_tensor_tensor DVE op
  - RoPE via partition-offset elementwise ops with a pre-signed sin table
  - causal attention in transposed-score form: S_T[tk,tq] = k_rot.T@q_rot,
    exp without max subtraction (logits bounded by the RMS norms),
    denominator via ones-matmul; its reciprocal is computed on the [1,CT]
    row and partition-broadcast on Pool (saves the K=1 PE matmul + the
    [128,CT] DVE reciprocal)
  - sigmoid gating fused with the softmax normalization (2 DVE ops)
  - partial o_proj: out_T[dout,t] = wo_slice.T @ gated (bf16 partials,
    summed in f32 on the host)

Scheduling notes (all engines execute their streams in order, so emission
order is the schedule):
  - DMA arrival is ordered to match consumption: small tables on the Pool
    SW queue; wk first on SP; hid tiles alternate SP/ACT in d order with
    the per-d weights (wv, head-0 q/gate blocks) on the opposite queue, so
    phase 0's d-outer loop paces with arrival; remaining q/gate heads and
    wo stream in behind
  - wqq/wqg are stored head-major in DRAM ([NHL*D, HD]) so head-0's
    16 d-tiles (needed in phase 0) are a contiguous early 1 MB
  - per chunk: projection pairs with norm/rope chains sandwiched between
    them, then attention with the two head-pairs' m-loops interleaved
  - o_proj of chunk c-1 (and for chunk 0, the next chunk's first
    projection pair) is drip-fed between attention m-steps as PE filler,
    finishing by ~80% of the m-loop so the PSUM drain clears before the
    next chunk's projections need the banks
  - the final chunk's o_proj is split into head pairs: the (h0,h1) half
    runs as fillers inside the second head-pair's m-loop (partials staged
    in SBUF bf16), so only the (h2,h3) half + add remains after the last
    attention step
  - sigmoids are explicitly ordered after the chunk's norm chains on ACT
    (a scheduler dependency) to avoid ACT function-table reload thrash
"""

import os
import sys
from contextlib import ExitStack

import numpy as np

sys.path.insert(0, "/opt/trn_rl_repo")

import ml_dtypes  # noqa: E402

import concourse.bass as bass  # noqa: E402
import concourse.mybir as mybir  # noqa: E402
import concourse.tile as tile  # noqa: E402
from concourse import bacc  # noqa: E402
from concourse import masks as masks_mod  # noqa: E402

F32 = mybir.dt.float32
BF16 = mybir.dt.bfloat16
F8 = mybir.dt.float8e4
AF = mybir.ActivationFunctionType
ALU = mybir.AluOpType
DR = mybir.MatmulPerfMode.DoubleRow
BF = ml_dtypes.bfloat16
F8NP = (ml_dtypes.float8_e4m3fn if hasattr(ml_dtypes, "float8_e4m3fn")
        else ml_dtypes.float8_e4m3)
GSC = 64.0               # fp8 gate-weight scale (escapes e4m3 subnormals)

P = 128
B, T, D = 2, 2048, 2048
NH, NKV, HD = 16, 4, 128
NHL = NH // NKV          # local q heads per core (4)
CH = 4                   # tq chunks
CT = T // CH             # 512 tokens per chunk
DT = D // P              # 16 contraction tiles
KT = T // P              # 16 tk tiles
EPS = 1e-6
SCALE = HD ** -0.5
N_CORES = 8


def _norm_rope(nc, pools, psr, ones_col, eps_t, x_bf, w_ap,
               cos_sl, sin_sl, out_ap, n):
    """RMS-norm (over partitions) + RoPE on a [128, n] channel-major tile.

    x_bf: [128, n] bf16 SBUF (pre-norm channels-on-partitions tile)
    w_ap: [128, 1] f32 norm weight
    cos_sl/sin_sl: [128, n] bf16 (sin pre-signed: rows 0-63 negated)
    out_ap: [128, n] bf16 destination
    """
    sbw, sbr = pools
    xsq = sbw.tile([P, n], BF16, tag="tmpa", name="xsq")
    nc.vector.tensor_tensor(xsq[:], x_bf, x_bf, op=ALU.mult)
    ssq = psr.tile([1, n], F32, tag="row", name="ssq")
    nc.tensor.matmul(ssq[:], ones_col, xsq[:], start=True, stop=True)
    rsq = sbr.tile([1, n], BF16, tag="rsq", name="rsq")
    absr = nc.scalar.activation(rsq[:], ssq[:], AF.Abs_reciprocal_sqrt,
                                scale=1.0 / HD, bias=eps_t)
    rbb = sbw.tile([P, n], BF16, tag="rbb", name="rbb", bufs=2)
    nc.gpsimd.partition_broadcast(rbb[:], rsq[:], channels=P)
    xn = sbw.tile([P, n], BF16, tag="xn", name="xn")
    nc.vector.scalar_tensor_tensor(xn[:], rbb[:], w_ap, x_bf,
                                   op0=ALU.mult, op1=ALU.mult)
    t1 = sbw.tile([P, n], BF16, tag="tmpb", name="t1")
    nc.vector.tensor_tensor(t1[:], xn[:], cos_sl, op=ALU.mult)
    h = HD // 2
    xs = sbw.tile([P, n], BF16, tag="tmpc", name="xs")
    nc.vector.tensor_copy(xs[0:h, :], xn[h:P, :])
    nc.vector.tensor_copy(xs[h:P, :], xn[0:h, :])
    t2 = sbw.tile([P, n], BF16, tag="tmpa", name="t2")
    nc.vector.tensor_tensor(t2[:], xs[:], sin_sl, op=ALU.mult)
    nc.vector.tensor_tensor(out_ap, t1[:], t2[:], op=ALU.add)
    return absr


def build_nc():
    nc = bacc.Bacc("TRN2", target_bir_lowering=False, debug=False,
                   num_devices=N_CORES)
    # All weights are host-prepacked into [128, n*128] partition-major
    # layouts so each loads with one (or two) big DMAs.
    hid_d = nc.dram_tensor("hid", [D, T], BF16, kind="ExternalInput")
    wqq_d = nc.dram_tensor("wqq", [P, NHL * DT * HD], BF16,
                           kind="ExternalInput")
    wqg_d = nc.dram_tensor("wqg", [P, DT * HD], BF16,
                           kind="ExternalInput")
    wqg8_d = nc.dram_tensor("wqg8", [P, NHL * 8 * 2 * HD], F8,
                            kind="ExternalInput")
    wk_d = nc.dram_tensor("wk", [P, DT * HD], BF16, kind="ExternalInput")
    wv_d = nc.dram_tensor("wv", [P, DT * HD], BF16, kind="ExternalInput")
    wo_d = nc.dram_tensor("wo", [P, NHL * D], BF16, kind="ExternalInput")
    cos_d = nc.dram_tensor("cost", [P, T], BF16, kind="ExternalInput")
    sin_d = nc.dram_tensor("sinpm", [P, T], BF16, kind="ExternalInput")
    qw_d = nc.dram_tensor("qw", [P, 1], F32, kind="ExternalInput")
    kw_d = nc.dram_tensor("kw", [P, 1], F32, kind="ExternalInput")
    mask_d = nc.dram_tensor("masks", [P, P], BF16, kind="ExternalInput")
    out_d = nc.dram_tensor("out_t", [D, T], BF16, kind="ExternalOutput")
    # final chunk's o_proj head-pair-0 partial; host adds it to out_t
    out0_d = nc.dram_tensor("out0", [D, CT], BF16, kind="ExternalOutput")

    with tile.TileContext(nc) as tc, ExitStack() as ctx, \
            nc.allow_low_precision(reason="bf16 softmax temps validated by rel_err"):
        sbp = ctx.enter_context(tc.tile_pool(name="sbp", bufs=1))
        sbw = ctx.enter_context(tc.tile_pool(name="sbw", bufs=3))
        sbr = ctx.enter_context(tc.tile_pool(name="sbr", bufs=2))
        sbq = ctx.enter_context(tc.tile_pool(name="sbq", bufs=6))
        psp = ctx.enter_context(tc.tile_pool(name="psp", bufs=2, space="PSUM"))
        pss = ctx.enter_context(tc.tile_pool(name="pss", bufs=2, space="PSUM"))
        psa = ctx.enter_context(tc.tile_pool(name="psa", bufs=2, space="PSUM"))
        psr = ctx.enter_context(tc.tile_pool(name="psr", bufs=2, space="PSUM"))

        # ---- persistent tiles + loads ----
        # Arrival order is engineered: tiny tables via the Pool SW queue;
        # wk first on SP; hid[d] alternates SP/ACT in d order with the per-d
        # small weights on the opposite queue; the rest streams in behind.
        qw = sbp.tile([P, 1], F32, tag="qw")
        nc.gpsimd.dma_start(qw[:], qw_d[:, :])
        kw = sbp.tile([P, 1], F32, tag="kw")
        nc.gpsimd.dma_start(kw[:], kw_d[:, :])
        masks = sbp.tile([P, P], BF16, tag="masks")
        nc.gpsimd.dma_start(masks[:], mask_d[:, :])
        cost = sbp.tile([P, T], BF16, tag="cost")
        nc.gpsimd.dma_start(cost[:], cos_d[:, :])
        sinpm = sbp.tile([P, T], BF16, tag="sinpm")
        nc.gpsimd.dma_start(sinpm[:], sin_d[:, :])

        # Packed weight tiles: one [128, n*128] tile per tensor, loaded with
        # few big DMAs (each dma_start costs ~0.7-1.3us of serialized ring
        # time regardless of size; one ring streams ~400 GB/s). Ring plan:
        #   SYNC:   wk, hid evens, wqq/wqg heads 1-3, wo  (then all outputs)
        #   SCALAR: wv, wqq/wqg head 0, hid odds, cost, sin
        #   POOL:   tiny tables
        wk_sb = sbp.tile([P, DT * HD], BF16, tag="wk_sb")
        nc.sync.dma_start(wk_sb[:], wk_d[:, :])
        wv_sb = sbp.tile([P, DT * HD], BF16, tag="wv_sb")
        nc.scalar.dma_start(wv_sb[:], wv_d[:, :])
        wqq_pk = sbp.tile([P, NHL * DT * HD], BF16, tag="wqq_pk")
        wqg_pk = sbp.tile([P, DT * HD], BF16, tag="wqg_pk")
        nc.scalar.dma_start(wqq_pk[:, 0:DT * HD], wqq_d[:, 0:DT * HD])
        nc.scalar.dma_start(wqg_pk[:], wqg_d[:, :])
        wqg8_pk = sbp.tile([P, NHL * 8, 2, HD], F8, tag="wqg8_pk")
        nc.scalar.dma_start(wqg8_pk[:], wqg8_d[:, :])
        hid = []
        for d in range(DT):
            t = sbp.tile([P, T], BF16, tag=f"hid{d}", name=f"hid{d}")
            hid.append(t)
        for d in range(0, DT, 2):
            nc.sync.dma_start(hid[d][:], hid_d[d * P:(d + 1) * P, :])
        for d in range(1, DT, 2):
            nc.scalar.dma_start(hid[d][:], hid_d[d * P:(d + 1) * P, :])
        nc.sync.dma_start(wqq_pk[:, DT * HD:], wqq_d[:, DT * HD:])
        wo_pk = sbp.tile([P, NHL * D], BF16, tag="wo_pk")
        nc.sync.dma_start(wo_pk[:, 0:2 * D], wo_d[:, 0:2 * D])
        nc.sync.dma_start(wo_pk[:, 2 * D:], wo_d[:, 2 * D:])

        def wk(d):
            return wk_sb[:, d * HD:(d + 1) * HD]

        def wv(d):
            return wv_sb[:, d * HD:(d + 1) * HD]

        def wqq_sl(h, d):
            return wqq_pk[:, (h * DT + d) * HD:(h * DT + d + 1) * HD]

        def wqg_sl(h, d):
            assert h == 0
            return wqg_pk[:, d * HD:(d + 1) * HD]

        def wqg8_sl(h, p):
            return wqg8_pk[:, h * 8 + p, :, :]

        def wo_sl(ct4, ds_):
            return wo_pk[:, ct4 * D + ds_.start:ct4 * D + ds_.stop]

        ones_col = sbp.tile([P, 1], BF16, tag="ones_col")
        nc.vector.memset(ones_col[:], 1.0)
        eps_t = sbp.tile([1, 1], F32, tag="eps_t")
        nc.vector.memset(eps_t[:], EPS)
        ident = sbp.tile([P, P], BF16, tag="ident")
        masks_mod.make_identity(nc, ident[:])
        krot = sbp.tile([P, T], BF16, tag="krot")
        vsb = []
        for i in range(KT):
            vsb.append(sbp.tile([P, HD], BF16, tag=f"v{i}", name=f"v{i}"))

        # ---- phase 0: everything d-outer so the PE paces with the hid DMA
        # stream. Per d: 4 k-proj, 2 v-proj (chunks 0-1), and chunk-0's
        # first q/gate projection pair (head-0 weight tiles arrive early).
        kps = [pss.tile([P, CT], F32, tag="ss", name="kps0"),
               pss.tile([P, CT], F32, tag="ss", name="kps1"),
               psa.tile([P, CT], F32, tag="aa", name="kps2"),
               psa.tile([P, CT], F32, tag="aa", name="kps3")]
        vps01 = [psr.tile([P, CT], F32, tag="row", name="vps0"),
                 psr.tile([P, CT], F32, tag="row", name="vps1")]
        qp0 = psp.tile([P, CT], F32, tag="pp", name="qp0")
        gp0 = psp.tile([P, CT], F32, tag="pp", name="gp0")
        cs0 = slice(0, CT)
        for d in range(DT):
            st, sp = (d == 0), (d == DT - 1)
            for c in range(CH):
                cs = slice(c * CT, (c + 1) * CT)
                nc.tensor.matmul(kps[c][:], wk(d), hid[d][:, cs],
                                 start=st, stop=sp)
            for c in range(2):
                cs = slice(c * CT, (c + 1) * CT)
                nc.tensor.matmul(vps01[c][:], wv(d), hid[d][:, cs],
                                 start=st, stop=sp)
            nc.tensor.matmul(qp0[:], wqq_sl(0, d), hid[d][:, cs0],
                             start=st, stop=sp)
            nc.tensor.matmul(gp0[:], wqg_sl(0, d), hid[d][:, cs0],
                             start=st, stop=sp)
        kbfs = []
        for c in range(CH):
            kbf = sbw.tile([P, CT], BF16, tag="kbf", name="kbf", bufs=4)
            nc.vector.tensor_copy(kbf[:], kps[c][:])
            kbfs.append(kbf)
        vct = sbp.tile([P, T], BF16, tag="vct")
        for c in range(2):
            cs = slice(c * CT, (c + 1) * CT)
            nc.vector.tensor_copy(vct[:, cs], vps01[c][:])
        q_sb0 = sbq.tile([P, CT], BF16, tag="q_sb", bufs=3, name="q_sb0")
        nc.vector.tensor_copy(q_sb0[:], qp0[:])
        g_sb0 = sbq.tile([P, CT], BF16, tag="g_sb", bufs=5, name="g_sb0")
        nc.vector.tensor_copy(g_sb0[:], gp0[:])
        pre_pairs = {0: (q_sb0, g_sb0)}
        for c in range(2, CH):
            cs = slice(c * CT, (c + 1) * CT)
            ps = psr.tile([P, CT], F32, tag="row", name="vcps")
            for d in range(DT):
                nc.tensor.matmul(ps[:], wv(d), hid[d][:, cs],
                                 start=(d == 0), stop=(d == DT - 1))
            nc.vector.tensor_copy(vct[:, cs], ps[:])
        for tt in range(KT):
            tps = pss.tile([P, P], BF16, tag="ss", name="tps")
            nc.tensor.transpose(tps[:], vct[:, tt * P:(tt + 1) * P],
                                ident[:])
            nc.vector.tensor_copy(vsb[tt][:], tps[:])

        # ---- phase 1: per tq-chunk: q/gate proj, attention ----
        # o_proj for chunk c-1 is emitted after chunk c's norm chains so the
        # PE has dense work while the chains' DVE/ACT latency drains.
        def _o_proj_pair1(og):
            """Final chunk: pair-1 accumulation (pair-0 went to out0_d)."""
            ocs = slice((CH - 1) * CT, CH * CT)
            pools4 = [(psp, "pp"), (pss, "ss"), (psa, "aa"), (psr, "row")]
            for dt in range(DT):
                ds_ = slice(dt * P, (dt + 1) * P)
                pl, tg = pools4[dt % 4]
                pso = pl.tile([P, CT], F32, tag=tg, name="pso")
                nc.tensor.matmul(pso[:], wo_sl(2, ds_), og[2][:],
                                 start=True, stop=False)
                nc.tensor.matmul(pso[:], wo_sl(3, ds_), og[3][:],
                                 start=False, stop=True)
                osb = sbw.tile([P, CT], BF16, tag="osb", bufs=2, name="osb")
                nc.vector.tensor_copy(osb[:], pso[:])
                nc.sync.dma_start(out_d[ds_, ocs], osb[:])

        prev_gated = None
        h8_map = {}

        def _h8(c, cs):
            if c not in h8_map:
                tiles = []
                for p in range(8):
                    t8 = sbw.tile([P, 2, CT], F8, tag="h8", bufs=8,
                                  name="h8")
                    nc.vector.tensor_copy(t8[:, 0, :], hid[2 * p][:, cs])
                    nc.vector.tensor_copy(t8[:, 1, :], hid[2 * p + 1][:, cs])
                    tiles.append(t8)
                h8_map[c] = tiles
            return h8_map[c]

        for c in range(CH):
            cs = slice(c * CT, (c + 1) * CT)
            q_sbs = []
            g_sbs = []
            sigs = []
            qrots = []
            fp8_gate = set()

            chain_absr = []

            def _q_chain(h, c=None, cs=None, q_sbs=None, qrots=None):
                qrot = sbw.tile([P, CT], BF16, tag="qrot", bufs=5,
                                name="qrot")
                a = _norm_rope(nc, (sbw, sbr), psr, ones_col[:], eps_t[:],
                               q_sbs[h][:], qw[:], cost[:, cs],
                               sinpm[:, cs], qrot[:], CT)
                qrots.append(qrot)
                chain_absr.append(a)

            for h in range(NHL):
                if h == 0 and c in pre_pairs:
                    q_sbs.append(pre_pairs[c][0])
                    g_sbs.append(pre_pairs[c][1])
                    chain_absr.append(_norm_rope(
                        nc, (sbw, sbr), psr, ones_col[:], eps_t[:],
                        kbfs[c][:], kw[:], cost[:, cs], sinpm[:, cs],
                        krot[:, cs], CT))
                    continue
                ps = psp.tile([P, CT], F32, tag="pp")
                for d in range(DT):
                    nc.tensor.matmul(ps[:], wqq_sl(h, d), hid[d][:, cs],
                                     start=(d == 0), stop=(d == DT - 1))
                q_sb = sbq.tile([P, CT], BF16, tag="q_sb", bufs=3)
                nc.vector.tensor_copy(q_sb[:], ps[:])
                q_sbs.append(q_sb)
                ps2 = psp.tile([P, CT], F32, tag="pp")
                h8t = _h8(c, cs)
                for p in range(8):
                    nc.tensor.matmul(ps2[:], wqg8_sl(h, p), h8t[p][:],
                                     start=(p == 0), stop=(p == 7),
                                     perf_mode=DR)
                g_sb = sbq.tile([P, CT], BF16, tag="g_sb", bufs=5)
                nc.vector.tensor_copy(g_sb[:], ps2[:])
                g_sbs.append(g_sb)
                fp8_gate.add(h)
                # sandwich a norm/rope chain after each proj pair so the
                # chain's DVE/ACT latency hides behind the next pair's mms
                if h == 0:
                    chain_absr.append(_norm_rope(
                        nc, (sbw, sbr), psr, ones_col[:], eps_t[:],
                        kbfs[c][:], kw[:], cost[:, cs], sinpm[:, cs],
                        krot[:, cs], CT))
                else:
                    _q_chain(h - 1, c=c, cs=cs, q_sbs=q_sbs, qrots=qrots)
            _q_chain(NHL - 1, c=c, cs=cs, q_sbs=q_sbs, qrots=qrots)
            for h in range(NHL):
                sig = sbq.tile([P, CT], BF16, tag="sig", bufs=4, name="sig")
                sc = 1.0 / GSC if h in fp8_gate else 1.0
                si = nc.scalar.activation(sig[:], g_sbs[h][:], AF.Sigmoid,
                                          scale=sc)
                # order sigmoids after the chunk's norm chains on ACT (each
                # function switch reloads the ACT table, ~1.3us)
                bass._add_dep_helper(si.ins, chain_absr[-1].ins, sync=False,
                                     reason="group sigmoids after absrsqrt")
                sigs.append(sig)
            gated = []
            nm = 4 * c + 4
            # Filler work drip-fed between attention m-steps keeps the PE
            # dense while ACT runs the exps: o_proj(c-1) tiles; for chunk 0
            # the next chunk's first projection pair; for the final chunk
            # its own o_proj pair-0 halves (during the hp=2 loop only).
            fillers = []
            if prev_gated is not None:
                ocs = slice((c - 1) * CT, c * CT)

                def _mk_oproj(dt, ocs=ocs, og=prev_gated):
                    def run():
                        ds_ = slice(dt * P, (dt + 1) * P)
                        pso = psp.tile([P, CT], F32, tag="pp", name="pso")
                        for ct4 in range(NHL):
                            nc.tensor.matmul(pso[:], wo_sl(ct4, ds_),
                                             og[ct4][:], start=(ct4 == 0),
                                             stop=(ct4 == NHL - 1))
                        osb = sbw.tile([P, CT], BF16, tag="osb", bufs=2,
                                       name="osb")
                        nc.vector.tensor_copy(osb[:], pso[:])
                        nc.sync.dma_start(out_d[ds_, ocs], osb[:])
                    return run
                fillers += [_mk_oproj(dt) for dt in range(DT)]
            if c == 0:
                cs1 = slice(CT, 2 * CT)
                qp1 = psp.tile([P, CT], F32, tag="pp", name="qp1")
                gp1 = psp.tile([P, CT], F32, tag="pp", name="gp1")

                def _mk_proj(ps_t, w_sl, dlist):
                    def run():
                        for d in dlist:
                            nc.tensor.matmul(
                                ps_t[:], w_sl(0, d), hid[d][:, cs1],
                                start=(d == 0), stop=(d == DT - 1))
                    return run
                for d0 in range(0, DT, 4):
                    fillers.append(_mk_proj(qp1, wqq_sl,
                                            range(d0, d0 + 4)))
                for d0 in range(0, DT, 4):
                    fillers.append(_mk_proj(gp1, wqg_sl,
                                            range(d0, d0 + 4)))

            # final-chunk pair-0 o_proj fillers (only valid inside hp=2)
            def _mk_pair0(dt):
                def run():
                    ds_ = slice(dt * P, (dt + 1) * P)
                    pso = psp.tile([P, CT], F32, tag="pp", name="pso0")
                    nc.tensor.matmul(pso[:], wo_sl(0, ds_), gated[0][:],
                                     start=True, stop=False)
                    nc.tensor.matmul(pso[:], wo_sl(1, ds_), gated[1][:],
                                     start=False, stop=True)
                    osb = sbw.tile([P, CT], BF16, tag="osb", bufs=2,
                                   name="osb0")
                    nc.vector.tensor_copy(osb[:], pso[:])
                    nc.sync.dma_start(out0_d[ds_, :], osb[:])
                return run

            fill = {"i": 0}
            n_steps = 2 * nm

            def _fill_tick(step):
                # finish fillers by ~80% of the m-steps so the last PSUM
                # drain clears before the next chunk's projections
                due = min(len(fillers),
                          len(fillers) * (step + 1) * 5 // (4 * n_steps) + 1)
                while fill["i"] < due:
                    fillers[fill["i"]]()
                    fill["i"] += 1

            step_no = [0]
            for hp in (0, 2):
                if c == CH - 1 and hp == 2:
                    fillers.extend(_mk_pair0(dt) for dt in range(DT))
                pair = (hp, hp + 1)
                denoms = {h: psr.tile([1, CT], F32, tag="row",
                                      name=f"denom{h}") for h in pair}
                attns = {h: psa.tile([P, CT], F32, tag="aa",
                                     name=f"attn{h}") for h in pair}
                for m in range(nm):
                    ks = slice(m * P, (m + 1) * P)
                    r = m - 4 * c
                    lo = P * r if r > 0 else 0
                    ns = slice(lo, CT)
                    for h in pair:
                        sps = pss.tile([P, CT], F32, tag="ss", name="sps")
                        nc.tensor.matmul(sps[:, ns], krot[:, ks],
                                         qrots[h][:, ns],
                                         start=True, stop=True)
                        E = sbw.tile([P, CT], BF16, tag="E", name="E",
                                     bufs=4)
                        nc.scalar.activation(E[:, ns], sps[:, ns], AF.Exp,
                                             scale=SCALE)
                        if r >= 0:
                            nc.vector.tensor_tensor(
                                E[:, lo:lo + P], E[:, lo:lo + P],
                                masks[:, 0:P], op=ALU.mult)
                        nc.tensor.matmul(denoms[h][:, ns], ones_col[:],
                                         E[:, ns], start=(m == 0),
                                         stop=(m == nm - 1))
                        nc.tensor.matmul(attns[h][:, ns], vsb[m][:],
                                         E[:, ns], start=(m == 0),
                                         stop=(m == nm - 1))
                    _fill_tick(step_no[0])
                    step_no[0] += 1
                for h in pair:
                    rcp = sbr.tile([1, CT], F32, tag="rcp", name="rcp")
                    rsc = sbr.tile([1, CT], F32, tag="rsc", name="rsc")
                    nc.vector.reciprocal_approx_accurate(
                        out=rcp[:], in_=denoms[h][:], scratch=rsc[:])
                    rcpb = sbr.tile([1, CT], BF16, tag="rcpb", name="rcpb")
                    nc.vector.tensor_copy(rcpb[:], rcp[:])
                    rcb = sbw.tile([P, CT], BF16, tag="rcb", name="rcb",
                                   bufs=2)
                    nc.gpsimd.partition_broadcast(rcb[:], rcpb[:],
                                                  channels=P)
                    tmp = sbw.tile([P, CT], BF16, tag="tmpc", name="tmp")
                    nc.vector.tensor_tensor(tmp[:], attns[h][:], sigs[h][:],
                                            op=ALU.mult)
                    g = sbq.tile([P, CT], BF16, tag="gated", bufs=8)
                    nc.vector.tensor_tensor(g[:], tmp[:], rcb[:],
                                            op=ALU.mult)
                    gated.append(g)

            while fill["i"] < len(fillers):
                fillers[fill["i"]]()
                fill["i"] += 1
            if c == 0:
                q_sb1 = sbq.tile([P, CT], BF16, tag="q_sb", bufs=3,
                                 name="q_sb1")
                nc.vector.tensor_copy(q_sb1[:], qp1[:])
                g_sb1 = sbq.tile([P, CT], BF16, tag="g_sb", bufs=5,
                                 name="g_sb1")
                nc.vector.tensor_copy(g_sb1[:], gp1[:])
                pre_pairs[1] = (q_sb1, g_sb1)
            prev_gated = gated
        _o_proj_pair1(prev_gated)
    nc.compile()
    return nc


def _pack_pm(w):
    """[n*128, m] -> [128, n*m]: stack the 128-row tiles along columns."""
    n = w.shape[0] // P
    return np.ascontiguousarray(
        w.reshape(n, P, -1).transpose(1, 0, 2).reshape(P, -1))


def _pack_f8(w):
    """[D, HD] -> [128, 8*2*HD] fp8: d-pair-packed DoubleRow layout."""
    w8 = w.astype(F8NP)
    return np.ascontiguousarray(
        w8.reshape(8, 2, P, HD).transpose(2, 0, 1, 3).reshape(P, -1))


def make_in_maps(hidden, cos, sin, wq, wk, wv, wo, q_norm_w, k_norm_w):
    """Build the 8 per-core input maps (host-side sharding + layout prep)."""
    i_idx = np.arange(P)[:, None]
    j_idx = np.arange(P)[None, :]
    masks = (j_idx >= i_idx).astype(BF)
    in_maps = []
    for core in range(N_CORES):
        b, g = core // NKV, core % NKV
        heads = range(NHL * g, NHL * g + NHL)
        g8 = NHL * g  # global index of local head 0
        sin_t = sin[b].T.copy()
        sin_t[:HD // 2] = -sin_t[:HD // 2]
        in_maps.append({
            "hid": np.ascontiguousarray(hidden[b].T).astype(BF),
            # packed [128, NHL*DT*HD]: head-major then d-tile-major columns
            "wqq": np.concatenate(
                [_pack_pm(wq[:, h * 2 * HD: h * 2 * HD + HD]) for h in heads],
                1).astype(BF),
            "wqg": _pack_pm(
                wq[:, g8 * 2 * HD + HD: (g8 + 1) * 2 * HD]).astype(BF),
            "wqg8": np.concatenate(
                [_pack_f8(wq[:, h * 2 * HD + HD: (h + 1) * 2 * HD] * GSC)
                 for h in heads], 1),
            "wk": _pack_pm(wk[:, g * HD:(g + 1) * HD]).astype(BF),
            "wv": _pack_pm(wv[:, g * HD:(g + 1) * HD]).astype(BF),
            "wo": _pack_pm(wo[NHL * HD * g: NHL * HD * (g + 1), :]).astype(BF),
            "cost": np.ascontiguousarray(cos[b].T).astype(BF),
            "sinpm": np.ascontiguousarray(sin_t).astype(BF),
            "qw": np.ascontiguousarray(q_norm_w[:, None]).astype(np.float32),
            "kw": np.ascontiguousarray(k_norm_w[:, None]).astype(np.float32),
            "masks": np.ascontiguousarray(masks),
        })
    return in_maps


def _install_ntff_hook():
    """Inject antenv.axon_hooks with a ctypes NTFF profile hook.

    The container's antenv package lacks axon_hooks, so bass_utils'
    trace=True path can't find the hook. Replicates the boot script's
    _ntff_profile_via_ctypes against libaxon_pjrt.so.
    """
    import contextlib
    import ctypes
    import types

    if "antenv.axon_hooks" in sys.modules:
        return
    lib = None
    for so_path in ("/opt/axon/libaxon_pjrt.so",
                    "/root/.axon_site/axon/libaxon_pjrt.so"):
        try:
            lib = ctypes.CDLL(so_path)
            break
        except OSError:
            continue
    if lib is None:
        return
    if not hasattr(lib, "axon_start_nrt_profile"):
        return
    lib.axon_start_nrt_profile.argtypes = [ctypes.POINTER(ctypes.c_int64),
                                           ctypes.c_size_t]
    lib.axon_start_nrt_profile.restype = ctypes.c_int64
    lib.axon_stop_nrt_profile.argtypes = [ctypes.c_char_p]
    lib.axon_stop_nrt_profile.restype = ctypes.c_int64

    @contextlib.contextmanager
    def _hook(output_dir, device_ids):
        import jax

        jax.devices()
        if device_ids:
            ids = (ctypes.c_int64 * len(device_ids))(*device_ids)
            rc = lib.axon_start_nrt_profile(ids, len(device_ids))
        else:
            rc = lib.axon_start_nrt_profile(None, 0)
        if rc != 0:
            raise RuntimeError(f"axon_start_nrt_profile rc={rc}")
        try:
            yield
        finally:
            n = lib.axon_stop_nrt_profile(str(output_dir).encode())
            print(f"profile: {n} file(s) written to {output_dir}",
                  file=sys.stderr)

    m = types.ModuleType("antenv.axon_hooks")
    m.get_axon_ntff_profile_hook = lambda: _hook
    m.set_axon_ntff_profile_hook = lambda h: None
    sys.modules["antenv.axon_hooks"] = m


_NC_CACHE = None


def _get_nc():
    global _NC_CACHE
    if _NC_CACHE is None:
        _NC_CACHE = build_nc()
    return _NC_CACHE


def kernel(hidden_BTD, cos_BTK, sin_BTK, wq, wk, wv, wo, q_norm_w, k_norm_w,
           segment_ids_BT=None, position_ids_BT=None, **_unused):
    from concourse.bass_utils import run_bass_kernel_spmd

    in_maps = make_in_maps(
        np.asarray(hidden_BTD, np.float32), np.asarray(cos_BTK, np.float32),
        np.asarray(sin_BTK, np.float32), np.asarray(wq, np.float32),
        np.asarray(wk, np.float32), np.asarray(wv, np.float32),
        np.asarray(wo, np.float32), np.asarray(q_norm_w, np.float32),
        np.asarray(k_norm_w, np.float32))
    nc = _get_nc()
    trace = bool(int(os.environ.get("BASS_KERNEL_TRACE", "0")))
    if trace:
        _install_ntff_hook()
    res = run_bass_kernel_spmd(nc, in_maps, core_ids=list(range(N_CORES)),
                               trace=trace)
    out = np.zeros((B, T, D), np.float32)
    for core in range(N_CORES):
        out[core // NKV] += res.results[core]["out_t"].astype(np.float32).T
        out[core // NKV][(CH - 1) * CT:] += \
            res.results[core]["out0"].astype(np.float32).T
    kernel.last_exec_time_ns = res.exec_time_ns
    kernel.last_results = res
    return out


kernel.last_exec_time_ns = None
kernel.last_results = None
